# revision 7
# baseline (speedup 1.0000x reference)
"""Trainium2 Bass kernel for ContrastiveMSELoss.

Reference computes, over all N^2 pairs (diagonal masked to 0):
    mse_ij  = (|x_i|^2 + |x_j|^2 - 2 x_i.x_j) / D
    sign_ij = +1 if class_i == class_j else -1
    loss    = mean_ij(sign_ij * mse_ij) + BETA

Using sum_{i,j in c} x_i.x_j = |M_c|^2 with M_c = sum_{i in c} x_i, the
loss collapses to class-bucketed first/second moments (O(N*D) work,
memory-bound -- no N x N gram matrix needed):

    T_same = sum_c (2 n_c SQ_c - 2 |M_c|^2) / D      (diag terms are 0)
    T_all  = (2 N SQ - 2 |M|^2) / D
    loss   = (2 T_same - T_all) / N^2 + BETA

Sharding: rows are split across 8 cores.  Each core packs [X | X^2] into a
bf16 [128, 512] rhs per 128-row chunk and one-hot class rows into the lhsT,
so a single accumulating matmul chain produces the partial per-class sums
M_c (cols 0..D-1) and per-dim squared sums (cols D..2D-1).  The host
combines the 8 partial [40, 512] outputs in float64.
"""

import numpy as np

import concourse.bacc as bacc
import concourse.bass as bass
import concourse.tile as tile
from concourse import mybir
from concourse.bass_utils import run_bass_kernel_spmd

N, D = 8192, 256
N_CORES = 8
ROWS = N // N_CORES          # 1024 rows per core
P = 128                      # partitions
CHUNKS = ROWS // P           # 8 chunks of 128 rows
NCLS = 40
BETA = 1.0
HALF = CHUNKS // 2           # chunks per pipeline half

_CACHE = {}


def _bcast(ap, pos, count):
    """Insert a zero-stride dim of size `count` at free-dim position `pos`."""
    pattern = [list(p) for p in ap.ap]
    pattern.insert(pos, [0, count])
    return bass.AP(tensor=ap.tensor, offset=ap.offset, ap=pattern)


def _build_bass():
    nc = bacc.Bacc(
        "TRN2",
        target_bir_lowering=False,
        debug=False,
        enable_asserts=True,
        num_devices=N_CORES,
    )
    x = nc.dram_tensor("x", [ROWS, D], mybir.dt.float32, kind="ExternalInput")
    # cls_grid[p, k] = class id (as f32) of shard row k*128 + p
    cls = nc.dram_tensor("cls", [P, CHUNKS], mybir.dt.float32, kind="ExternalInput")
    # iota[p, c] = c (host-supplied constant, avoids gpsimd iota + cast)
    iota = nc.dram_tensor("iota", [P, NCLS], mybir.dt.float32, kind="ExternalInput")
    # stats[c, :D] = sum of rows with class c; stats[c, D:] = per-dim x^2 sums
    stats = nc.dram_tensor(
        "stats", [NCLS, 2 * D], mybir.dt.float32, kind="ExternalOutput"
    )

    with tile.TileContext(nc) as tc:
        with (
            tc.tile_pool(name="work", bufs=1) as work,
            tc.tile_pool(name="psum", bufs=1, space="PSUM") as psum_pool,
        ):
            iota_sb = work.tile([P, NCLS], mybir.dt.float32, tag="iota_sb")
            nc.scalar.dma_start(out=iota_sb, in_=iota[:, :])
            cls_sb = work.tile([P, CHUNKS], mybir.dt.float32, tag="cls_sb")
            nc.scalar.dma_start(out=cls_sb, in_=cls[:, :])

            # raw f32 input, 2 chunks per DMA, issue split across sync/scalar
            xf = work.tile([P, CHUNKS, D], mybir.dt.float32, tag="xf")
            for j in range(CHUNKS // 2):
                eng = nc.sync if j % 2 == 0 else nc.scalar
                src = x[j * 2 * P : (j + 1) * 2 * P, :].rearrange(
                    "(k p) d -> p k d", p=P
                )
                eng.dma_start(out=xf[:, 2 * j : 2 * j + 2, :], in_=src)

            # bf16 matmul operands: [X | X^2] and one-hot classes
            xb = work.tile([P, CHUNKS, 2 * D], mybir.dt.bfloat16, tag="xb")
            oh = work.tile([P, CHUNKS, NCLS], mybir.dt.bfloat16, tag="oh")
            acc = psum_pool.tile([NCLS, 2 * D], mybir.dt.float32, tag="acc")

            for h in range(2):
                lo, hi = h * HALF, (h + 1) * HALF
                # cast X -> bf16 (DVE)
                nc.vector.tensor_copy(xb[:, lo:hi, :D], xf[:, lo:hi, :])
                # X^2 -> bf16 (ACT, keeps DVE free)
                nc.scalar.activation(
                    out=xb[:, lo:hi, D:],
                    in_=xf[:, lo:hi, :],
                    func=mybir.ActivationFunctionType.Square,
                )
                # one-hot: oh[p, k, c] = (cls[p, k] == c), one broadcast op
                nc.vector.tensor_tensor(
                    out=oh[:, lo:hi, :],
                    in0=_bcast(cls_sb[:, lo:hi], 2, NCLS),
                    in1=_bcast(iota_sb, 1, HALF),
                    op=mybir.AluOpType.is_equal,
                )
                for k in range(lo, hi):
                    nc.tensor.matmul(
                        acc,
                        oh[:, k, :],
                        xb[:, k, :],
                        start=(k == 0),
                        stop=(k == CHUNKS - 1),
                    )

            out_sb = work.tile([NCLS, 2 * D], mybir.dt.float32, tag="out_sb")
            nc.vector.tensor_copy(out_sb, acc)
            nc.sync.dma_start(out=stats[:, :], in_=out_sb)

    return nc


def _get_nc():
    if "nc" not in _CACHE:
        nc = _build_bass()
        nc.finalize()
        _CACHE["nc"] = nc
    return _CACHE["nc"]


_IOTA = np.ascontiguousarray(
    np.broadcast_to(np.arange(NCLS, dtype=np.float32), (P, NCLS))
)


def run_device(output, classes, **spmd_kwargs):
    """Run the per-core Bass kernel; returns (list of per-core stats, results)."""
    x = np.ascontiguousarray(np.asarray(output), dtype=np.float32)
    cls_f = np.asarray(classes).astype(np.float32)
    in_maps = []
    for s in range(N_CORES):
        xs = x[s * ROWS : (s + 1) * ROWS]
        cs = cls_f[s * ROWS : (s + 1) * ROWS]
        # cls_grid[p, k] = class of shard row k*128 + p
        cls_grid = np.ascontiguousarray(cs.reshape(CHUNKS, P).T)
        in_maps.append({"x": xs, "cls": cls_grid, "iota": _IOTA})
    res = run_bass_kernel_spmd(
        _get_nc(), in_maps, core_ids=list(range(N_CORES)), **spmd_kwargs
    )
    stats = [res.results[s]["stats"] for s in range(N_CORES)]
    return stats, res


def _combine(stats, classes):
    """Combine per-core partial class stats into the scalar loss (float64)."""
    tot = np.sum(np.asarray(stats, dtype=np.float64), axis=0)  # [NCLS, 2D]
    M_c = tot[:, :D]                                           # class sums
    SQ_c = tot[:, D:].sum(axis=1)                              # class |x|^2 sums
    n_c = np.bincount(np.asarray(classes).astype(np.int64), minlength=NCLS).astype(
        np.float64
    )
    SQ = SQ_c.sum()
    M = M_c.sum(axis=0)
    T_same = (2.0 * (n_c * SQ_c).sum() - 2.0 * (M_c * M_c).sum()) / D
    T_all = (2.0 * N * SQ - 2.0 * (M @ M)) / D
    loss = (2.0 * T_same - T_all) / (float(N) * float(N)) + BETA
    return np.float32(loss)


def kernel(output, classes):
    stats, _ = run_device(output, classes)
    return _combine(stats, classes)


# revision 9
# speedup vs baseline: 1.0143x; 1.0143x over previous
"""Trainium2 Bass kernel for ContrastiveMSELoss.

Reference computes, over all N^2 pairs (diagonal masked to 0):
    mse_ij  = (|x_i|^2 + |x_j|^2 - 2 x_i.x_j) / D
    sign_ij = +1 if class_i == class_j else -1
    loss    = mean_ij(sign_ij * mse_ij) + BETA

Using sum_{i,j in c} x_i.x_j = |M_c|^2 with M_c = sum_{i in c} x_i, the
loss collapses to class-bucketed first/second moments (O(N*D) work,
memory-bound -- no N x N gram matrix needed):

    T_same = sum_c (2 n_c SQ_c - 2 |M_c|^2) / D      (diag terms are 0)
    T_all  = (2 N SQ - 2 |M|^2) / D
    loss   = (2 T_same - T_all) / N^2 + BETA

Sharding: rows are split across 8 cores.  Each core packs [X | X^2] into a
bf16 [128, 512] rhs per 128-row chunk and one-hot class rows into the lhsT,
so a single accumulating matmul chain produces the partial per-class sums
M_c (cols 0..D-1) and per-dim squared sums (cols D..2D-1).  The host
combines the 8 partial [40, 512] outputs in float64.
"""

import numpy as np

import concourse.bacc as bacc
import concourse.bass as bass
import concourse.tile as tile
from concourse import mybir
from concourse.bass_utils import run_bass_kernel_spmd

N, D = 8192, 256
N_CORES = 8
ROWS = N // N_CORES          # 1024 rows per core
P = 128                      # partitions
CHUNKS = ROWS // P           # 8 chunks of 128 rows
NCLS = 40
BETA = 1.0
HALF = CHUNKS // 2           # chunks per pipeline half

_CACHE = {}


def _bcast(ap, pos, count):
    """Insert a zero-stride dim of size `count` at free-dim position `pos`."""
    pattern = [list(p) for p in ap.ap]
    pattern.insert(pos, [0, count])
    return bass.AP(tensor=ap.tensor, offset=ap.offset, ap=pattern)


def _build_bass():
    nc = bacc.Bacc(
        "TRN2",
        target_bir_lowering=False,
        debug=False,
        enable_asserts=True,
        num_devices=N_CORES,
    )
    x = nc.dram_tensor("x", [ROWS, D], mybir.dt.float32, kind="ExternalInput")
    # cls_grid[p, k] = class id (as f32) of shard row k*128 + p
    cls = nc.dram_tensor("cls", [P, CHUNKS], mybir.dt.float32, kind="ExternalInput")
    # iota[p, c] = c (host-supplied constant, avoids gpsimd iota + cast)
    iota = nc.dram_tensor("iota", [P, NCLS], mybir.dt.float32, kind="ExternalInput")
    # stats[c, :D] = sum of rows with class c; stats[c, D:] = per-dim x^2 sums
    stats = nc.dram_tensor(
        "stats", [NCLS, 2 * D], mybir.dt.float32, kind="ExternalOutput"
    )

    with tile.TileContext(nc) as tc:
        with (
            tc.tile_pool(name="work", bufs=1) as work,
            tc.tile_pool(name="psum", bufs=1, space="PSUM") as psum_pool,
        ):
            iota_sb = work.tile([P, NCLS], mybir.dt.float32, tag="iota_sb")
            nc.gpsimd.dma_start(out=iota_sb, in_=iota[:, :])
            cls_sb = work.tile([P, CHUNKS], mybir.dt.float32, tag="cls_sb")
            nc.gpsimd.dma_start(out=cls_sb, in_=cls[:, :])

            # raw f32 input: one 128-row chunk per DMA so each lands on its
            # own HW queue; split issue between the two HWDGE engines
            xf = work.tile([P, CHUNKS, D], mybir.dt.float32, tag="xf")
            dma_engs = [nc.sync, nc.scalar]
            for k in range(CHUNKS):
                dma_engs[k % 2].dma_start(
                    out=xf[:, k, :], in_=x[k * P : (k + 1) * P, :]
                )

            # bf16 matmul operands: [X | X^2] and one-hot classes
            xb = work.tile([P, CHUNKS, 2 * D], mybir.dt.bfloat16, tag="xb")
            oh = work.tile([P, CHUNKS, NCLS], mybir.dt.bfloat16, tag="oh")
            acc = psum_pool.tile([NCLS, 2 * D], mybir.dt.float32, tag="acc")

            for h in range(2):
                lo, hi = h * HALF, (h + 1) * HALF
                # one-hot: oh[p, k, c] = (cls[p, k] == c), one broadcast op
                nc.vector.tensor_tensor(
                    out=oh[:, lo:hi, :],
                    in0=_bcast(cls_sb[:, lo:hi], 2, NCLS),
                    in1=_bcast(iota_sb, 1, HALF),
                    op=mybir.AluOpType.is_equal,
                )
            for k in range(CHUNKS):
                # cast X -> bf16 (DVE) and X^2 -> bf16 (ACT), per chunk so
                # the matmul chain pipelines against staggered DMA arrivals
                nc.vector.tensor_copy(xb[:, k, :D], xf[:, k, :])
                nc.scalar.activation(
                    out=xb[:, k, D:],
                    in_=xf[:, k, :],
                    func=mybir.ActivationFunctionType.Square,
                )
                nc.tensor.matmul(
                    acc,
                    oh[:, k, :],
                    xb[:, k, :],
                    start=(k == 0),
                    stop=(k == CHUNKS - 1),
                )

            out_sb = work.tile([NCLS, 2 * D], mybir.dt.float32, tag="out_sb")
            nc.vector.tensor_copy(out_sb, acc)
            nc.sync.dma_start(out=stats[:, :], in_=out_sb)

    return nc


def _get_nc():
    if "nc" not in _CACHE:
        nc = _build_bass()
        nc.finalize()
        _CACHE["nc"] = nc
    return _CACHE["nc"]


_IOTA = np.ascontiguousarray(
    np.broadcast_to(np.arange(NCLS, dtype=np.float32), (P, NCLS))
)


def run_device(output, classes, **spmd_kwargs):
    """Run the per-core Bass kernel; returns (list of per-core stats, results)."""
    x = np.ascontiguousarray(np.asarray(output), dtype=np.float32)
    cls_f = np.asarray(classes).astype(np.float32)
    in_maps = []
    for s in range(N_CORES):
        xs = x[s * ROWS : (s + 1) * ROWS]
        cs = cls_f[s * ROWS : (s + 1) * ROWS]
        # cls_grid[p, k] = class of shard row k*128 + p
        cls_grid = np.ascontiguousarray(cs.reshape(CHUNKS, P).T)
        in_maps.append({"x": xs, "cls": cls_grid, "iota": _IOTA})
    res = run_bass_kernel_spmd(
        _get_nc(), in_maps, core_ids=list(range(N_CORES)), **spmd_kwargs
    )
    stats = [res.results[s]["stats"] for s in range(N_CORES)]
    return stats, res


def _combine(stats, classes):
    """Combine per-core partial class stats into the scalar loss (float64)."""
    tot = np.sum(np.asarray(stats, dtype=np.float64), axis=0)  # [NCLS, 2D]
    M_c = tot[:, :D]                                           # class sums
    SQ_c = tot[:, D:].sum(axis=1)                              # class |x|^2 sums
    n_c = np.bincount(np.asarray(classes).astype(np.int64), minlength=NCLS).astype(
        np.float64
    )
    SQ = SQ_c.sum()
    M = M_c.sum(axis=0)
    T_same = (2.0 * (n_c * SQ_c).sum() - 2.0 * (M_c * M_c).sum()) / D
    T_all = (2.0 * N * SQ - 2.0 * (M @ M)) / D
    loss = (2.0 * T_same - T_all) / (float(N) * float(N)) + BETA
    return np.float32(loss)


def kernel(output, classes):
    stats, _ = run_device(output, classes)
    return _combine(stats, classes)
